# revision 52
# baseline (speedup 1.0000x reference)
"""GQA attention (B=4,S=2048,D=2048,H=16,KH=4) + RoPE + causal mask on 8 trn2 cores.

Sharding: 8 cores = 4 batches x 2 head-groups. Group g owns the 8 q-heads with
h%4 in {2g, 2g+1}, so each core computes K/V for only its 2 kv heads (no
duplicated K/V work between the two cores of a batch). Each core runs
attention for its 8 heads over all 2048 q rows with block-causal skipping and
a partial output projection; the host sums the two fp16 partials per batch.

Per-core pipeline (fp16 matmuls, fp32 accumulate/softmax; fp8 was tried and
rejected: each fp8 stage in the q/k/v/probs path adds ~4% output error
because attention averaging shrinks y and its noise equally):
  x fp16 arrives host-transposed, 512-col-blocked -> xT [d, s] resident
  K/V projections (2 kv heads) -> rope(K) -> kT [hd, s] fp16; V [s128, kb, kv, hd|1]
  rope uses a DVE stream_shuffle for the pair swap (no PE matmul, no ACT copy)
  per (qc, h): Q proj -> rope -> qT [hd, 2048]
  per head, per q-chunk qc (512 wide):
    off-band kb < 4qc: scoresT = kT-block^T @ qT-chunk; exp from PSUM (ACT)
    band kb = 4qc+sb: one matmul over cols [sb*128, 512); exp; the 128-wide
      diagonal sub-block gets a multiplicative 0/1 triangle mask on DVE
    AV: y[q, hd|sum] = sum_kb probsT_kb^T @ [V|1], skipping fully-masked kbs;
    normalize; PE-transpose
  out_partial[q, dm] = sum_{local h} yT_h^T @ wo_h -> fp16 (host adds pairs)

DMA: x/weights stream on the sync HWDGE ring in consumption order; constants
ride the scalar ring in parallel (transfers on one ring serialize).
"""
import math

import numpy as np

B, S, D = 4, 2048, 2048
H, KH, HD = 16, 4, 128
HL = 8                   # q heads per core
KHL = 2                  # kv heads per core
DC = D // 128            # contraction chunks
NKB = S // 128           # key blocks
NQC = S // 512           # q chunks
NCORES = 8
SCALE = 1.0 / math.sqrt(HD)
EXP_BIAS = -4.0
N_WARM = 90              # HAM warm-up matmuls bridging the DMA head

# adjacent-pair swap within each 32-lane quadrant (rope rotate-half)
SWAP_MASK = [i ^ 1 for i in range(32)]

_cache = {}


def _build(causal: bool):
    import concourse.bacc as bacc
    import concourse.tile as tile
    import concourse.mybir as mybir

    f16, f32 = mybir.dt.float16, mybir.dt.float32
    Act = mybir.ActivationFunctionType

    nc = bacc.Bacc("TRN2", target_bir_lowering=False, debug=False,
                   num_devices=NCORES)

    # all inputs host-pre-arranged partition-major so each DMA lands with
    # >=2KB contiguous per partition (big descriptors -> ~line-rate HBM)
    # xt[p, sc, dc, n] = x[b].T[dc*128+p, sc*512+n]
    xt = nc.dram_tensor("xt", [128, NQC, DC, 512], f16,
                        kind="ExternalInput").ap()
    wq_d = nc.dram_tensor("wqg", [128, 2, DC, 4 * HD], f16,
                          kind="ExternalInput").ap()
    wk_d = nc.dram_tensor("wkg", [128, KHL, DC, HD], f16,
                          kind="ExternalInput").ap()
    wv_d = nc.dram_tensor("wvg", [128, DC, KHL * HD], f16,
                          kind="ExternalInput").ap()
    wo_d = nc.dram_tensor("wog", [128, DC // 2, 4, 512], f16,
                          kind="ExternalInput").ap()
    # causal: 0/1 multiplicative triangle for the diagonal 128x128 sub-blocks.
    # general: additive mask in pre-scale score units (clamped to +-1e4; exp
    # underflow to exactly 0 matches the reference's exp(-1e9)), [p, kb, q].
    mshape = [128, 128] if causal else [128, NKB, S]
    maskt = nc.dram_tensor("maskt", mshape, f16, kind="ExternalInput").ap()
    c2 = nc.dram_tensor("c2", [128, S], f16, kind="ExternalInput").ap()
    s2 = nc.dram_tensor("s2", [128, S], f16, kind="ExternalInput").ap()
    ident = nc.dram_tensor("ident", [128, 128], f16, kind="ExternalInput").ap()
    outp = nc.dram_tensor("outp", [S, D], f16, kind="ExternalOutput").ap()

    VOFF = KHL * HD          # wv columns inside wkv

    with tile.TileContext(nc) as tc:
        with tc.tile_pool(name="const", bufs=1) as constp, \
             tc.tile_pool(name="resid", bufs=1) as resid, \
             tc.tile_pool(name="psA", bufs=1, space="PSUM") as psA:
            identt = constp.tile([128, 128], f16)
            mtrit = constp.tile([128, 128], f16)
            bias_t = constp.tile([128, 1], f32)
            nc.vector.memset(bias_t, EXP_BIAS)

            # HAM warm-up: the PE clock sits at 1.2 GHz until ~3.4us of
            # sustained activity. The first ~10us are DMA-bound, so run
            # throwaway matmuls through the head window; real matmuls
            # then start at 2.4 GHz with at most one 128-col MM of delay.
            warm = constp.tile([128, 128], f16)
            nc.vector.memset(warm, 0.0)
            warmP = psA.tile([128, 512], f32, name="warmP", tag="trp",
                             bufs=1)

            def pe_fill(k):
                # elastic filler: absorbs DMA-arrival jitter when the next
                # chain's data is late; costs ~54ns/matmul (warm) otherwise
                for _ in range(k):
                    nc.tensor.matmul(warmP[:, 0:128], warm, warm, start=True,
                                     stop=True)

            pe_fill(N_WARM)

            kT = resid.tile([128, KHL, S], f16)           # [hd, kv, s]
            V = resid.tile([128, NKB, KHL, HD + 1], f16)  # [s128, kb, kv, hd|1]
            qTs = resid.tile([128, HL, S], f16)           # [hd, h, s]
            for kb in range(NKB):
                nc.vector.memset(V[:, kb, :, HD:HD + 1], 1.0)

            # two pools: the big weight/x tiles sit below the rope temps, so
            # the attention pools (opened after both close) reuse the weight
            # region -- whose last readers are PE matmuls -- instead of the
            # rope temps still being drained by DVE/GpSimd at the boundary.
            with tc.tile_pool(name="p_w", bufs=1) as p_w, \
                 tc.tile_pool(name="p_x", bufs=1) as p_x:
                xT = p_w.tile([128, NQC, DC, 512], f16)   # [d128, sc, dc, n]
                wkt = p_w.tile([128, KHL, DC, HD], f16)
                wvt = p_w.tile([128, DC, KHL * HD], f16)
                wqt = p_w.tile([128, 2, DC, 4 * HD], f16)
                c2t = p_x.tile([128, S], f16)
                s2t = p_x.tile([128, S], f16)
                def x_dma(sc, lo, hi, eng=None):
                    (eng or nc.sync).dma_start(out=xT[:, sc, lo:hi],
                                               in_=xt[:, sc, lo:hi])

                # consts on the scalar ring; x/weights stream on the sync
                # ring in consumption order (early DMA has a ~7us
                # completion-latency floor, so ordering is all that matters)
                nc.scalar.dma_start(out=identt, in_=ident)
                if causal:
                    nc.scalar.dma_start(out=mtrit, in_=maskt)
                nc.scalar.dma_start(out=c2t, in_=c2)
                nc.scalar.dma_start(out=s2t, in_=s2)

                x_dma(0, 0, 4)
                nc.sync.dma_start(out=wkt[:, 0, 0:8], in_=wk_d[:, 0, 0:8])
                nc.sync.dma_start(out=wkt[:, 1, 0:8], in_=wk_d[:, 1, 0:8])
                x_dma(0, 4, 8)
                nc.sync.dma_start(out=wkt[:, 0, 8:16], in_=wk_d[:, 0, 8:16])
                nc.sync.dma_start(out=wkt[:, 1, 8:16], in_=wk_d[:, 1, 8:16])
                x_dma(0, 8, 12)
                x_dma(0, 12, 16)
                nc.sync.dma_start(out=wvt[:, 0:8], in_=wv_d[:, 0:8])
                nc.sync.dma_start(out=wvt[:, 8:16], in_=wv_d[:, 8:16])
                x_dma(1, 0, 8)
                x_dma(1, 8, 16)
                nc.sync.dma_start(out=wqt[:, 0], in_=wq_d[:, 0])
                x_dma(2, 0, 8)
                x_dma(2, 8, 16)
                nc.sync.dma_start(out=wqt[:, 1], in_=wq_d[:, 1])
                x_dma(3, 0, 8)
                x_dma(3, 8, 16)

                def rope_evict(pP, out_ap, off, ncols, tag):
                    """out = pP*c2 + pairswap(pP)*s2, table cols [off, off+ncols)."""
                    qsw = p_x.tile([128, 512], f32, name=f"qsw_{tag}",
                                   tag="qsw", bufs=2)
                    nc.vector.stream_shuffle(qsw[:, 0:ncols], pP, SWAP_MASK)
                    m1 = p_x.tile([128, 512], f32, name=f"m1_{tag}", tag="m1", bufs=2)
                    m2 = p_x.tile([128, 512], f32, name=f"m2_{tag}", tag="m2", bufs=2)
                    nc.vector.tensor_mul(m1[:, 0:ncols], pP, c2t[:, off:off + ncols])
                    nc.vector.tensor_mul(m2[:, 0:ncols], qsw[:, 0:ncols],
                                         s2t[:, off:off + ncols])
                    nc.gpsimd.tensor_add(out_ap, m1[:, 0:ncols], m2[:, 0:ncols])

                # ---- Phase 1: K/V projections ----
                # the two K chains interleave per-dc so the DMA-paced first
                # block has twice the PE work per arriving x chunk
                for sc in range(4):
                    cs = slice(sc * 512, (sc + 1) * 512)
                    kPs = [psA.tile([128, 512], f32, name=f"kP{sc}_{kv}",
                                    tag="big", bufs=5) for kv in range(KHL)]
                    lo = 0
                    if sc == 0:
                        # sc0 start is DMA-gated: kv-sequential dc 0:4 needs
                        # only x0[0:4]+wk[kv] (first MMs fire one transfer
                        # earlier); interleaved kv from dc 4 on as usual
                        lo = 4
                        for kv in range(KHL):
                            for dc in range(lo):
                                nc.tensor.matmul(kPs[kv],
                                                 wkt[:, kv, dc, :],
                                                 xT[:, sc, dc, :],
                                                 start=(dc == 0), stop=False)
                    for dc in range(lo, DC):
                        for kv in range(KHL):
                            nc.tensor.matmul(kPs[kv],
                                             wkt[:, kv, dc, :],
                                             xT[:, sc, dc, :], start=(dc == 0),
                                             stop=(dc == DC - 1))
                    for kv in range(KHL):
                        rope_evict(kPs[kv], kT[:, kv, cs], sc * 512, 512,
                                   f"k{sc}_{kv}")
                    for sb in range(4):
                        kb = sc * 4 + sb
                        vP = psA.tile([128, 512], f32, name=f"vP{kb}",
                                      tag="big", bufs=5)
                        for dc in range(DC):
                            nc.tensor.matmul(
                                vP[:, 0:KHL * HD],
                                xT[:, sc, dc, sb * 128:(sb + 1) * 128],
                                wvt[:, dc, :],
                                start=(dc == 0), stop=(dc == DC - 1))
                        nc.scalar.copy(
                            out=V[:, kb, :, 0:HD],
                            in_=vP[:, 0:KHL * HD].rearrange(
                                "p (kv h) -> p kv h", kv=KHL))

                # ---- Phase 2: Q projections + rope, qc-outer ----
                for qc in range(NQC):
                    for h in range(HL):
                        # last two chains use the idle aux tag so the first
                        # attention matmuls don't WAR-wait on the rope
                        # pipeline still reading the big-tag PSUM bufs
                        last2 = qc == NQC - 1 and h >= HL - 2
                        qP = psA.tile([128, 512], f32, name=f"qP{h}_{qc}",
                                      tag="aux" if last2 else "big",
                                      bufs=2 if last2 else 5)
                        for dc in range(DC):
                            nc.tensor.matmul(
                                qP, wqt[:, h // 4, dc,
                                        (h % 4) * HD:(h % 4 + 1) * HD],
                                xT[:, qc, dc, :],
                                start=(dc == 0), stop=(dc == DC - 1))
                        rope_evict(qP, qTs[:, h, qc * 512:(qc + 1) * 512],
                                   qc * 512, 512, f"q{h}_{qc}")

            # ---- Phase 3: attention; Phase 4: output projection ----
            with tc.tile_pool(name="p_att", bufs=1) as ph, \
                 tc.tile_pool(name="p_4", bufs=1) as p4:
                wot = p4.tile([128, DC // 2, 4, 512], f16)  # [hd128, h, dmc, dm]
                nc.sync.dma_start(out=wot, in_=wo_d)

                def out_proj(qc, yTsb, last=False):
                    # output projection for one q-chunk (all local heads).
                    # On the final chunk ACT is idle, so alternate the PSUM
                    # evictions across both engines to shorten the tail.
                    for qsl in range(4):
                        qs = qc * 4 + qsl
                        osb = p4.tile([128, D], f16, name=f"osb{qs}",
                                      tag="osb", bufs=3)
                        for dmc in range(4):
                            oP = psA.tile([128, 512], f32, name=f"oP{qs}_{dmc}",
                                          tag="big", bufs=5)
                            for h in range(HL):
                                nc.tensor.matmul(
                                    oP, yTsb[:, h, qsl * 128:(qsl + 1) * 128],
                                    wot[:, h, dmc, :],
                                    start=(h == 0), stop=(h == HL - 1))
                            dsl = slice(dmc * 512, (dmc + 1) * 512)
                            if last and qsl == 3 and dmc == 3:
                                # final chunk: split eviction across both
                                # engines so the last DMA starts sooner
                                nc.vector.tensor_copy(
                                    out=osb[:, 1536:1792], in_=oP[:, 0:256])
                                nc.scalar.copy(
                                    out=osb[:, 1792:2048], in_=oP[:, 256:512])
                                nc.sync.dma_start(
                                    out=outp[qs * 128:(qs + 1) * 128,
                                             1536:1792],
                                    in_=osb[:, 1536:1792])
                                nc.sync.dma_start(
                                    out=outp[qs * 128:(qs + 1) * 128,
                                             1792:2048],
                                    in_=osb[:, 1792:2048])
                                continue
                            if last and dmc % 2 == 1:
                                nc.scalar.copy(out=osb[:, dsl], in_=oP)
                            else:
                                nc.vector.tensor_copy(out=osb[:, dsl], in_=oP)
                            if last:
                                # per-chunk DMAs so the final row's tail is
                                # only one 128KB transfer after the last evict
                                nc.sync.dma_start(
                                    out=outp[qs * 128:(qs + 1) * 128, dsl],
                                    in_=osb[:, dsl])
                        if not last:
                            nc.sync.dma_start(
                                out=outp[qs * 128:(qs + 1) * 128, :], in_=osb)

                def emit_scores(qc, h, probs, kbs, mqc, lo, hi):
                    kv = h % KHL
                    for j, kb in list(enumerate(kbs))[lo:hi]:
                        sc_ps = psA.tile([128, 512], f32, name=f"sc{h}_{qc}_{kb}",
                                         tag="big", bufs=5)
                        kslice = kT[:, kv, kb * 128:(kb + 1) * 128]
                        if causal and kb >= 4 * qc:
                            # band block: only cols [off, 512) are live;
                            # the first 128 are the diagonal sub-block.
                            off = (kb - 4 * qc) * 128
                            q0 = qc * 512 + off
                            nc.tensor.matmul(sc_ps[:, off:512], kslice,
                                             qTs[:, h, q0:(qc + 1) * 512],
                                             start=True, stop=True)
                            nc.scalar.activation(out=probs[:, j, off:512],
                                                 in_=sc_ps[:, off:512],
                                                 func=Act.Exp, bias=bias_t,
                                                 scale=SCALE)
                            nc.vector.tensor_mul(probs[:, j, off:off + 128],
                                                 probs[:, j, off:off + 128],
                                                 mtrit)
                        else:
                            masked = not causal
                            nc.tensor.matmul(sc_ps, kslice,
                                             qTs[:, h, qc * 512:(qc + 1) * 512],
                                             start=True, stop=not masked)
                            if masked:
                                # accumulate the additive mask on the PE
                                nc.tensor.matmul(sc_ps, identt, mqc[:, kb, :],
                                                 start=False, stop=True)
                            nc.scalar.activation(out=probs[:, j, :], in_=sc_ps,
                                                 func=Act.Exp, bias=bias_t,
                                                 scale=SCALE)

                def av_qs(qc, h, probs, kbs, qs):
                    kv = h % KHL
                    jmax = 4 * qc + qs + 1 if causal else len(kbs)
                    yP = psA.tile([128, HD + 1], f32, name=f"yP{h}_{qc}_{qs}",
                                  tag="aux", bufs=2)
                    for j in range(jmax):
                        nc.tensor.matmul(yP,
                                         probs[:, j, qs * 128:(qs + 1) * 128],
                                         V[:, kbs[j], kv, :], start=(j == 0),
                                         stop=(j == jmax - 1))
                    rc = ph.tile([128, 1], f32, name=f"rc{h}_{qc}_{qs}",
                                 tag="rc", bufs=4)
                    nc.vector.reciprocal(rc, yP[:, HD:HD + 1])
                    ysb = ph.tile([128, HD], f16, name=f"ysb{h}_{qc}_{qs}",
                                  tag="ysb", bufs=8)
                    nc.vector.tensor_scalar_mul(ysb, yP[:, 0:HD], rc)
                    return ysb

                def tr_emit(qc, h, ysb, qs, yTsb):
                    # transpose as a regular matmul (y.T @ I); emitted one AV
                    # group late so it never stalls on the fresh recip/mul
                    # DVE chain of the ysb it reads
                    yTp = psA.tile([128, 512], f32, name=f"yTp{h}_{qc}_{qs}",
                                   tag="trp", bufs=1)
                    nc.tensor.matmul(yTp[:, 0:128], ysb, identt,
                                     start=True, stop=True)
                    nc.vector.tensor_copy(
                        out=yTsb[:, h, qs * 128:(qs + 1) * 128],
                        in_=yTp[:, 0:128])

                def op_chunks(qc, yTsb):
                    # out_proj as 16 chunk thunks to spread across heads
                    osbs = {}

                    def mk(qsl, dmc):
                        def go():
                            qs = qc * 4 + qsl
                            if dmc == 0:
                                osbs[qsl] = p4.tile([128, D], f16,
                                                    name=f"osb{qs}",
                                                    tag="osb", bufs=3)
                            osb = osbs[qsl]
                            oP = psA.tile([128, 512], f32, name=f"oP{qs}_{dmc}",
                                          tag="big", bufs=5)
                            for h in range(HL):
                                nc.tensor.matmul(
                                    oP, yTsb[:, h, qsl * 128:(qsl + 1) * 128],
                                    wot[:, h, dmc, :],
                                    start=(h == 0), stop=(h == HL - 1))
                            dsl = slice(dmc * 512, (dmc + 1) * 512)
                            nc.vector.tensor_copy(out=osb[:, dsl], in_=oP)
                            if dmc == 3:
                                nc.sync.dma_start(
                                    out=outp[qs * 128:(qs + 1) * 128, :],
                                    in_=osb)
                        return go

                    return [mk(qsl, dmc) for qsl in range(4) for dmc in range(4)]

                # software pipeline: interleave score groups of head h with
                # the AV chains of head h-1 and out_proj chunks of the
                # previous q-chunk, so the in-order PE queue always has work
                # while the ACT exp stream catches up
                pending = None
                tr_q = []            # deferred last-transpose of a head
                for qc in range(NQC):
                    yTsb = p4.tile([128, HL, 512], f16, name=f"yTsb{qc}",
                                   tag="yTsb", bufs=2)
                    mqc = None
                    if not causal:
                        mqc = ph.tile([128, NKB, 512], f16, name=f"mqc{qc}",
                                      tag="mqc", bufs=2)
                        nc.sync.dma_start(out=mqc,
                                          in_=maskt[:, :, qc * 512:(qc + 1) * 512])
                    kbs = list(range(4 * qc + 4)) if causal else list(range(NKB))
                    n = len(kbs)
                    bounds = [n * i // 4 for i in range(5)]
                    prev = None
                    opq = []
                    for h in range(HL):
                        probs = ph.tile([128, 16, 512], f16, name=f"pr{h}_{qc}",
                                        tag="probs", bufs=3)
                        if h == 0 and pending is not None:
                            opq = op_chunks(*pending)
                            pending = None
                        ysbs = []
                        for gi in range(4):
                            emit_scores(qc, h, probs, kbs, mqc,
                                        bounds[gi], bounds[gi + 1])
                            if tr_q:
                                tr_q.pop(0)()
                            if prev is not None:
                                ysbs.append(av_qs(qc, prev[0], prev[1], kbs, gi))
                                if gi >= 1:
                                    tr_emit(qc, prev[0], ysbs[gi - 1], gi - 1,
                                            yTsb)
                            else:
                                # h==0: no AV to interleave; fill the exp
                                # latency with an out_proj chunk instead
                                if opq:
                                    opq.pop(0)()
                        if prev is not None:
                            hh, ysb3 = prev[0], ysbs[3]
                            tr_q.append(lambda hh=hh, ysb3=ysb3, qc=qc,
                                        yTsb=yTsb: tr_emit(qc, hh, ysb3, 3,
                                                           yTsb))
                        # filler spread: keep some out_proj chunks for the
                        # late, exp-paced heads instead of draining early
                        npop = 0 if h == 0 else (2 if h <= 5 else 1)
                        for _ in range(min(npop, len(opq))):
                            opq.pop(0)()
                        prev = (h, probs)
                    ysbs = []
                    for qs in range(4):
                        ysbs.append(av_qs(qc, prev[0], prev[1], kbs, qs))
                        if tr_q:
                            tr_q.pop(0)()
                        if qs >= 1:
                            tr_emit(qc, prev[0], ysbs[qs - 1], qs - 1, yTsb)
                    hh, ysb3 = prev[0], ysbs[3]
                    tr_q.append(lambda hh=hh, ysb3=ysb3, qc=qc,
                                yTsb=yTsb: tr_emit(qc, hh, ysb3, 3, yTsb))
                    while opq:
                        opq.pop(0)()

                    pending = (qc, yTsb)
                while tr_q:
                    tr_q.pop(0)()
                if pending is not None:
                    out_proj(*pending, last=True)

    nc.compile()
    return nc


def _host_prep(x, wq, wk, wv, wo, freqs_cos, freqs_sin, mask, causal):
    f16 = np.float16
    id_np = np.eye(128, dtype=f16)
    sign = np.tile(np.array([-1.0, 1.0], np.float32), 64)[:, None]
    c2_np = np.ascontiguousarray(np.repeat(freqs_cos.T, 2, axis=0).astype(f16))
    s2_np = np.ascontiguousarray(
        (np.repeat(freqs_sin.T, 2, axis=0) * sign).astype(f16))

    if causal:
        # 0/1 triangle (key p kept when p <= query q) for the diagonal blocks
        p = np.arange(128)[:, None]
        q = np.arange(128)[None, :]
        mt = (p <= q).astype(f16)
    else:
        mt = np.clip(mask.astype(np.float64) / SCALE, -1e4, 1e4).astype(f16)
        mt = mt.reshape(NKB, 128, S).transpose(1, 0, 2)
    mt = np.ascontiguousarray(mt)

    shared = {"maskt": mt, "c2": c2_np, "s2": s2_np, "ident": id_np}
    # x partition-major: xh[p, sc, dc, n] = x[b][sc*512+n, dc*128+p]
    xb = [np.ascontiguousarray(
        x[b].astype(f16).reshape(NQC, 512, DC, 128).transpose(3, 0, 2, 1))
        for b in range(B)]
    # group g owns q heads with h%KH in {2g, 2g+1} -> kv heads {2g, 2g+1}
    hg = [[h for h in range(H) if h % KH in (2 * g, 2 * g + 1)]
          for g in range(2)]
    # wq: [128, half, dc, 512];  wk: [128, kv, dc, HD];  wv: [128, dc, 256]
    # wo: [128, h, dmc, 512] -- all with per-partition-contiguous layout
    wqg = [np.ascontiguousarray(np.concatenate(
        [wq[:, h * HD:(h + 1) * HD] for h in hg[g]], axis=1).astype(f16)
        .reshape(DC, 128, 2, 4 * HD).transpose(1, 2, 0, 3))
        for g in range(2)]
    wog = [np.ascontiguousarray(np.concatenate(
        [wo[h * HD:(h + 1) * HD, :] for h in hg[g]], axis=0).astype(f16)
        .reshape(HL, 128, 4, 512).transpose(1, 0, 2, 3))
        for g in range(2)]
    wkg = [np.ascontiguousarray(
        wk[:, 2 * g * HD:(2 * g + 2) * HD].astype(f16)
        .reshape(DC, 128, KHL, HD).transpose(1, 2, 0, 3))
        for g in range(2)]
    wvg = [np.ascontiguousarray(
        wv[:, 2 * g * HD:(2 * g + 2) * HD].astype(f16)
        .reshape(DC, 128, KHL * HD).transpose(1, 0, 2))
        for g in range(2)]
    in_maps = []
    for core in range(NCORES):
        b, g = core // 2, core % 2
        in_maps.append({"xt": xb[b], "wqg": wqg[g], "wog": wog[g],
                        "wkg": wkg[g], "wvg": wvg[g], **shared})
    return in_maps


def _is_causal(mask: np.ndarray) -> bool:
    if mask.shape != (S, S):
        return False
    iu = np.triu_indices(S, k=1)
    if not np.all(mask[iu] <= -1e8):
        return False
    il = np.tril_indices(S, k=0)
    return bool(np.all(mask[il] == 0.0))


def run(x, wq, wk, wv, wo, freqs_cos, freqs_sin, mask, trace=False):
    from concourse.bass_utils import run_bass_kernel_spmd

    causal = _is_causal(np.asarray(mask))
    key = "causal" if causal else "general"
    if key not in _cache:
        _cache[key] = _build(causal)
    nc = _cache[key]

    in_maps = _host_prep(
        np.asarray(x, np.float32), np.asarray(wq, np.float32),
        np.asarray(wk, np.float32), np.asarray(wv, np.float32),
        np.asarray(wo, np.float32), np.asarray(freqs_cos, np.float32),
        np.asarray(freqs_sin, np.float32), np.asarray(mask, np.float32), causal)

    res = run_bass_kernel_spmd(nc, in_maps, list(range(NCORES)), trace=trace)

    out = np.empty((B, S, D), dtype=np.float32)
    for b in range(B):
        out[b] = (res.results[2 * b]["outp"].astype(np.float32)
                  + res.results[2 * b + 1]["outp"].astype(np.float32))
    return out, res


def kernel(x, wq, wk, wv, wo, freqs_cos, freqs_sin, mask):
    out, _ = run(x, wq, wk, wv, wo, freqs_cos, freqs_sin, mask, trace=False)
    return out

